# revision 9
# baseline (speedup 1.0000x reference)
# Trainium2 Bass kernel for nn_CrossAttention_16664473109002.
# Data-parallel over batch: 16 batches -> 8 NeuronCores (2 per core).
#
# Per-batch pipeline (per core):
#   q = dwconv3x3(x_q, wq, bq)   [512, 4096] f32   (DVE stt chain + ACT center tap)
#   k = dwconv3x3(x_kv, wk, bk)  [128, 4096] f32
#   v = dwconv3x3(x_kv, wv, bv)  [128, 4096] f32
#   q,k split into exact bf16 hi/lo pairs (ACT cast + GpSimd subtract),
#       DMA-xbar-transposed SBUF->SBUF to [n, c] / [n, k] layouts
#   aw_t[k,c] = sum_n kT qT / sqrt(128): 3-term Karatsuba bf16 on TensorE
#       (kh*qh + kh*ql + kl*qh, f32 PSUM accumulation => f32-grade logits)
#   softmax over k (partition axis): Exp on ACT (scale folds 1/sqrt(K)),
#       denominator via ones-matmul, reciprocal, broadcast-matmul, normalize
#   top-51-of-512 along c per (b,k) row: 7 rounds max8/match_replace8;
#       threshold = 51st largest; mask fused as (aw >= thr) * aw in one stt
#   attn = awm^T @ v in bf16 on TensorE; out = x_q + attn (stt from PSUM)
#   aw output transposed [k,c] -> [c,k] on TensorE.
import sys
import numpy as np

sys.path.insert(0, "/opt/trn_rl_repo")

B, C, K, H, W = 16, 512, 128, 64, 64
N = H * W                      # 4096
NCORES = 8
BPC = B // NCORES              # batches per core
CB = C // 128                  # 4 channel blocks
PADF = 2 + 66 * 64             # 4226 padded free size; interior at [65, 4161)
INT0 = 65
NKEEP = int(0.1 * C)           # 51
SCALE = 1.0 / float(np.sqrt(K))

_CACHE = {}


def _build():
    import concourse.bacc as bacc
    import concourse.tile as tile
    from concourse import mybir

    dt = mybir.dt
    A = mybir.AluOpType
    AF = mybir.ActivationFunctionType

    nc = bacc.Bacc("TRN2", target_bir_lowering=False, debug=False,
                   num_devices=NCORES)

    xq_d = nc.dram_tensor("x_q", [BPC, C, H, W], dt.float32, kind="ExternalInput").ap()
    xkv_d = nc.dram_tensor("x_kv", [BPC, K, H, W], dt.float32, kind="ExternalInput").ap()
    wq_d = nc.dram_tensor("wq", [C, 9], dt.float32, kind="ExternalInput").ap()
    bq_d = nc.dram_tensor("bq", [C, 1], dt.float32, kind="ExternalInput").ap()
    wk_d = nc.dram_tensor("wk", [K, 9], dt.float32, kind="ExternalInput").ap()
    bk_d = nc.dram_tensor("bk", [K, 1], dt.float32, kind="ExternalInput").ap()
    wv_d = nc.dram_tensor("wv", [K, 9], dt.float32, kind="ExternalInput").ap()
    bv_d = nc.dram_tensor("bv", [K, 1], dt.float32, kind="ExternalInput").ap()
    id_d = nc.dram_tensor("ident", [128, 128], dt.float32, kind="ExternalInput").ap()
    out_d = nc.dram_tensor("out", [BPC, C, H, W], dt.float32, kind="ExternalOutput").ap()
    aw_d = nc.dram_tensor("aw", [BPC, C, K], dt.float32, kind="ExternalOutput").ap()

    TAPS = [(dh, dw) for dh in (-1, 0, 1) for dw in (-1, 0, 1)]

    def tap_off(dh, dw):
        return INT0 + 64 * dh + dw

    with tile.TileContext(nc) as tc:
        with (
            tc.tile_pool(name="const", bufs=1) as cpool,
            tc.tile_pool(name="xq", bufs=2) as xqp,
            tc.tile_pool(name="xkv", bufs=1) as xkvp,
            tc.tile_pool(name="cv", bufs=2) as cvp,
            tc.tile_pool(name="hl", bufs=1) as hlp,
            tc.tile_pool(name="qT", bufs=1) as qTp,
            tc.tile_pool(name="kT", bufs=1) as kTp,
            tc.tile_pool(name="vb", bufs=1) as vbp,
            tc.tile_pool(name="sm", bufs=1) as smp,
            tc.tile_pool(name="xqa", bufs=1) as xqap,
            tc.tile_pool(name="psA", bufs=1, space="PSUM") as psA,
            tc.tile_pool(name="psB", bufs=2, space="PSUM") as psB,
            tc.tile_pool(name="psC", bufs=1, space="PSUM") as psC,
        ):
            # ---- constants ----
            wq_t = cpool.tile([128, CB, 9], dt.float32, tag="wq")
            nwq_t = cpool.tile([128, CB, 9], dt.float32, tag="nwq")
            bq_t = cpool.tile([128, CB], dt.float32, tag="bq")
            for cb in range(CB):
                nc.sync.dma_start(wq_t[:, cb], wq_d[cb * 128:(cb + 1) * 128])
                nc.sync.dma_start(bq_t[:, cb:cb + 1], bq_d[cb * 128:(cb + 1) * 128])
            wk_t = cpool.tile([128, 9], dt.float32, tag="wk")
            nwk_t = cpool.tile([128, 9], dt.float32, tag="nwk")
            bk_t = cpool.tile([128, 1], dt.float32, tag="bk")
            wv_t = cpool.tile([128, 9], dt.float32, tag="wv")
            nwv_t = cpool.tile([128, 9], dt.float32, tag="nwv")
            bv_t = cpool.tile([128, 1], dt.float32, tag="bv")
            nc.sync.dma_start(wk_t[:], wk_d[:])
            nc.sync.dma_start(bk_t[:], bk_d[:])
            nc.sync.dma_start(wv_t[:], wv_d[:])
            nc.sync.dma_start(bv_t[:], bv_d[:])
            nc.vector.tensor_scalar(nwq_t[:], wq_t[:], -1.0, None, op0=A.mult)
            nc.vector.tensor_scalar(nwk_t[:], wk_t[:], -1.0, None, op0=A.mult)
            nc.vector.tensor_scalar(nwv_t[:], wv_t[:], -1.0, None, op0=A.mult)
            id32 = cpool.tile([128, 128], dt.float32, tag="id32")
            nc.sync.dma_start(id32[:], id_d[:])
            ones_col = cpool.tile([128, 1], dt.float32, tag="ones_col")
            nc.scalar.activation(ones_col[:], bk_t[:], AF.Copy, bias=1.0, scale=0.0)
            ones_row = cpool.tile([1, 128], dt.float32, tag="ones_row")
            nc.scalar.activation(ones_row[:], id32[:1, :], AF.Copy, bias=1.0, scale=0.0)

            def load_padded(pool, src_ap, tag):
                t = pool.tile([128, PADF], dt.float32, tag=tag)
                nc.gpsimd.memset(t[:, 0:INT0], 0.0)
                nc.gpsimd.memset(t[:, INT0 + N:PADF], 0.0)
                nc.sync.dma_start(t[:, INT0:INT0 + N],
                                  src_ap.rearrange("c h w -> c (h w)"))
                return t

            def conv(xt, w_ap_fn, nw_ap_fn, b_ap, out_t):
                """3x3 depthwise conv from padded tile xt into out_t [128, N]."""
                y = out_t[:]
                nc.scalar.activation(y, xt[:, tap_off(0, 0):tap_off(0, 0) + N],
                                     AF.Identity, bias=b_ap, scale=w_ap_fn(4))
                for (dh, dw) in TAPS:
                    if (dh, dw) == (0, 0):
                        continue
                    ti = (dh + 1) * 3 + (dw + 1)
                    nc.vector.scalar_tensor_tensor(
                        y, xt[:, tap_off(dh, dw):tap_off(dh, dw) + N],
                        w_ap_fn(ti), y, op0=A.mult, op1=A.add)
                # wrap fixups: remove wrongly-added row-wrap terms at w=0/63
                yv = out_t[:].rearrange("p (h w) -> p h w", w=64)
                for dh in (-1, 0, 1):
                    ti = (dh + 1) * 3 + 0
                    src = xt[:, 64 + 64 * dh: 64 + 64 * dh + N].rearrange(
                        "p (h w) -> p h w", w=64)[:, :, 0]
                    nc.vector.scalar_tensor_tensor(
                        yv[:, :, 0], src, nw_ap_fn(ti), yv[:, :, 0],
                        op0=A.mult, op1=A.add)
                    ti = (dh + 1) * 3 + 2
                    s0 = 129 + 64 * dh - 63   # element j of [:, :, 63] = s0+64j+63
                    src = xt[:, s0: s0 + N].rearrange(
                        "p (h w) -> p h w", w=64)[:, :, 63]
                    nc.vector.scalar_tensor_tensor(
                        yv[:, :, 63], src, nw_ap_fn(ti), yv[:, :, 63],
                        op0=A.mult, op1=A.add)

            def split_transpose(src_t, outh, outl, ch0):
                """f32 [128,N] -> exact bf16 (hi,lo), quarter-wise, DMA-xbar
                transposed into outh/outl[:, ch0+8q : ch0+8q+8, :]."""
                for qd in range(4):
                    sl = src_t[:, qd * 1024:(qd + 1) * 1024]
                    h_t = hlp.tile([128, 1024], dt.bfloat16, tag="h")
                    l_t = hlp.tile([128, 1024], dt.bfloat16, tag="l")
                    nc.scalar.activation(h_t[:], sl, AF.Copy, bias=0.0, scale=1.0)
                    nc.gpsimd.tensor_tensor(l_t[:], sl, h_t[:], op=A.subtract)
                    c0 = ch0 + qd * 8
                    nc.scalar.dma_start_transpose(outh[:, c0:c0 + 8], h_t[:])
                    nc.scalar.dma_start_transpose(outl[:, c0:c0 + 8], l_t[:])

            for b in range(BPC):
                # ---- x_kv load, k/v convs (x_kv freed before q phase) ----
                xkv_t = load_padded(xkvp, xkv_d[b], "xkv")
                k_t = cvp.tile([128, N], dt.float32, tag="cv")
                conv(xkv_t, lambda i: wk_t[:, i:i + 1],
                     lambda i: nwk_t[:, i:i + 1], bk_t[:], k_t)
                kTh = kTp.tile([128, 32, 128], dt.bfloat16, tag="kTh")
                kTl = kTp.tile([128, 32, 128], dt.bfloat16, tag="kTl")
                split_transpose(k_t, kTh, kTl, 0)

                v_t = cvp.tile([128, N], dt.float32, tag="cv")
                conv(xkv_t, lambda i: wv_t[:, i:i + 1],
                     lambda i: nwv_t[:, i:i + 1], bv_t[:], v_t)
                v_b = vbp.tile([128, N], dt.bfloat16, tag="vb")
                nc.scalar.activation(v_b[:], v_t[:], AF.Copy, bias=0.0, scale=1.0)

                # ---- q convs + splits + transposes ----
                qTh = qTp.tile([128, 32, C], dt.bfloat16, tag="qTh")
                qTl = qTp.tile([128, 32, C], dt.bfloat16, tag="qTl")
                for cb in range(CB):
                    xq_t = load_padded(xqp, xq_d[b, cb * 128:(cb + 1) * 128], "xq")
                    q_t = cvp.tile([128, N], dt.float32, tag="cv")
                    conv(xq_t, lambda i, cb=cb: wq_t[:, cb, i:i + 1],
                         lambda i, cb=cb: nwq_t[:, cb, i:i + 1],
                         bq_t[:, cb:cb + 1], q_t)
                    split_transpose(q_t, qTh[:, :, cb * 128:(cb + 1) * 128],
                                    qTl[:, :, cb * 128:(cb + 1) * 128], 0)

                # prefetch x_q for the attention-output add (independent DMAs)
                xq_reloads = []
                for cb in range(CB):
                    xq_f = xqap.tile([128, N], dt.float32, tag="xqa")
                    nc.scalar.dma_start(xq_f[:], xq_d[b, cb * 128:(cb + 1) * 128]
                                        .rearrange("c h w -> c (h w)"))
                    xq_reloads.append(xq_f)

                # ---- aw matmul: 3-term Karatsuba bf16 accumulated in PSUM ----
                awt_ps = psA.tile([128, C], dt.float32, tag="awt")
                terms = [(kTh, qTh), (kTh, qTl), (kTl, qTh)]
                nmm = len(terms) * 32
                i = 0
                for (kt, qt) in terms:
                    for ch in range(32):
                        nc.tensor.matmul(awt_ps[:], kt[:, ch], qt[:, ch],
                                         start=(i == 0), stop=(i == nmm - 1))
                        i += 1

                # ---- softmax over k (partition axis) ----
                et = smp.tile([128, C], dt.float32, tag="et")
                nc.scalar.activation(et[:], awt_ps[:], AF.Exp, bias=0.0, scale=SCALE)
                den_ps = psC.tile([1, C], dt.float32, tag="den")
                nc.tensor.matmul(den_ps[:], ones_col[:], et[:], start=True, stop=True)
                rec = smp.tile([1, C], dt.float32, tag="rec")
                nc.vector.reciprocal(rec[:], den_ps[:])
                bc_ps = psC.tile([128, C], dt.float32, tag="bc")
                nc.tensor.matmul(bc_ps[:], ones_row[:], rec[:], start=True, stop=True)
                awn = smp.tile([128, C], dt.float32, tag="awn")
                nc.vector.tensor_tensor(awn[:], et[:], bc_ps[:], op=A.mult)

                # ---- top-51 along c per k row ----
                scratch = smp.tile([128, C], dt.float32, tag="scratch")
                nc.vector.tensor_copy(scratch[:], awn[:])
                m8 = smp.tile([128, 7, 8], dt.float32, tag="rec")
                for r in range(7):
                    nc.vector.max(m8[:, r], scratch[:])
                    if r < 6:
                        nc.vector.match_replace(scratch[:], m8[:, r], scratch[:], -1e30)
                thr = m8[:, 6, 2:3]      # rank 51 (round 6 holds ranks 49..56)
                awm = smp.tile([128, C], dt.float32, tag="scratch")
                nc.vector.scalar_tensor_tensor(awm[:], awn[:], thr, awn[:],
                                               op0=A.is_ge, op1=A.mult)
                awm_b = smp.tile([128, C], dt.bfloat16, tag="et")
                nc.scalar.activation(awm_b[:], awm[:], AF.Copy, bias=0.0, scale=1.0)

                # ---- attention output: attn = awm^T @ v ; out = x_q + attn ----
                for cb in range(CB):
                    xq_f = xq_reloads[cb]
                    for hp in range(4):
                        at_ps = psB.tile([128, 1024], dt.float32, tag="attn")
                        for bank in range(2):
                            o0 = hp * 1024 + bank * 512
                            nc.tensor.matmul(
                                at_ps[:, bank * 512:(bank + 1) * 512],
                                awm_b[:, cb * 128:(cb + 1) * 128],
                                v_b[:, o0:o0 + 512], start=True, stop=True)
                        nc.vector.scalar_tensor_tensor(
                            xq_f[:, hp * 1024:(hp + 1) * 1024], at_ps[:], 1.0,
                            xq_f[:, hp * 1024:(hp + 1) * 1024],
                            op0=A.mult, op1=A.add)
                    nc.scalar.dma_start(out_d[b, cb * 128:(cb + 1) * 128]
                                        .rearrange("c h w -> c (h w)"), xq_f[:])

                # ---- aw output: transpose [k,c] -> [c,k] ----
                for cb in range(CB):
                    tr_ps = psC.tile([128, 128], dt.float32, tag="awtr")
                    nc.tensor.transpose(tr_ps[:], awm[:, cb * 128:(cb + 1) * 128],
                                        id32[:])
                    awo = smp.tile([128, 128], dt.float32, tag="awn")
                    nc.scalar.activation(awo[:], tr_ps[:], AF.Copy, bias=0.0, scale=1.0)
                    nc.scalar.dma_start(aw_d[b, cb * 128:(cb + 1) * 128], awo[:])

    nc.compile()
    return nc


def _get_runner():
    if "runner" in _CACHE:
        return _CACHE["runner"]
    import jax
    from jax.sharding import Mesh, PartitionSpec
    from jax.experimental.shard_map import shard_map
    from concourse import mybir, bass2jax

    nc = _build()
    bass2jax.install_neuronx_cc_hook()
    partition_name = nc.partition_id_tensor.name if nc.partition_id_tensor else None
    in_names, out_names, out_avals = [], [], []
    for alloc in nc.m.functions[0].allocations:
        if not isinstance(alloc, mybir.MemoryLocationSet):
            continue
        name = alloc.memorylocations[0].name
        if alloc.kind == "ExternalInput":
            if name != partition_name:
                in_names.append(name)
        elif alloc.kind == "ExternalOutput":
            out_names.append(name)
            out_avals.append(jax.core.ShapedArray(
                tuple(alloc.tensor_shape), mybir.dt.np(alloc.dtype)))
    n_params, n_outs = len(in_names), len(out_avals)
    all_in_names = list(in_names) + list(out_names)
    if partition_name is not None:
        all_in_names.append(partition_name)
    donate = tuple(range(n_params, n_params + n_outs))

    def _body(*args):
        operands = list(args)
        if partition_name is not None:
            operands.append(bass2jax.partition_id_tensor())
        return tuple(bass2jax._bass_exec_p.bind(
            *operands, out_avals=tuple(out_avals), in_names=tuple(all_in_names),
            out_names=tuple(out_names), lowering_input_output_aliases=(),
            sim_require_finite=True, sim_require_nnan=True, nc=nc))

    devices = jax.devices()[:NCORES]
    mesh = Mesh(np.asarray(devices), ("core",))
    specs = (PartitionSpec("core"),)
    fn = jax.jit(
        shard_map(_body, mesh=mesh, in_specs=specs * (n_params + n_outs),
                  out_specs=specs * n_outs, check_rep=False),
        donate_argnums=donate, keep_unused=True)

    runner = {"fn": fn, "in_names": in_names, "out_names": out_names,
              "out_avals": out_avals}
    _CACHE["runner"] = runner
    return runner


def make_in_maps(x_q, x_kv, wq, bq, wk, bk, wv, bv):
    f32 = np.float32
    wq2 = np.ascontiguousarray(np.asarray(wq).reshape(C, 9), dtype=f32)
    wk2 = np.ascontiguousarray(np.asarray(wk).reshape(K, 9), dtype=f32)
    wv2 = np.ascontiguousarray(np.asarray(wv).reshape(K, 9), dtype=f32)
    bq2 = np.ascontiguousarray(np.asarray(bq).reshape(C, 1), dtype=f32)
    bk2 = np.ascontiguousarray(np.asarray(bk).reshape(K, 1), dtype=f32)
    bv2 = np.ascontiguousarray(np.asarray(bv).reshape(K, 1), dtype=f32)
    ident = np.eye(128, dtype=f32)
    maps = []
    for c in range(NCORES):
        maps.append({
            "x_q": np.ascontiguousarray(x_q[c * BPC:(c + 1) * BPC], dtype=f32),
            "x_kv": np.ascontiguousarray(x_kv[c * BPC:(c + 1) * BPC], dtype=f32),
            "wq": wq2, "bq": bq2, "wk": wk2, "bk": bk2, "wv": wv2, "bv": bv2,
            "ident": ident,
        })
    return maps


def run_on_cores(in_maps):
    r = _get_runner()
    args = [np.concatenate([np.asarray(m[n]) for m in in_maps], axis=0)
            for n in r["in_names"]]
    args += [np.zeros((NCORES * a.shape[0], *a.shape[1:]), a.dtype)
             for a in r["out_avals"]]
    outs = r["fn"](*args)
    per_core = []
    for c in range(NCORES):
        per_core.append(
            {n: np.asarray(outs[i]).reshape(NCORES, *r["out_avals"][i].shape)[c]
             for i, n in enumerate(r["out_names"])})
    return per_core


def kernel(x_q, x_kv, wq, bq, wk, bk, wv, bv):
    in_maps = make_in_maps(np.asarray(x_q), np.asarray(x_kv), np.asarray(wq),
                           np.asarray(bq), np.asarray(wk), np.asarray(bk),
                           np.asarray(wv), np.asarray(bv))
    per_core = run_on_cores(in_maps)
    out = np.concatenate([p["out"] for p in per_core], axis=0)
    aw = np.concatenate([p["aw"] for p in per_core], axis=0)
    return out, aw
